# revision 1
# baseline (speedup 1.0000x reference)
"""Causal attention (weight-normalized projections) Trainium2 Bass kernel.

Full-input contract: kernel(**inputs) takes the unsharded tensors from
setup_inputs() and returns the full [8, 32, 32, 512] output. Internally the
batch dim (8) is sharded 1:1 across 8 NeuronCores (data parallel); each core
runs an identical Bass program on its own batch.

Math per batch b:
  qf = query[b].reshape(1024, 256); kf = key[b].reshape(1024, 512)
  q = qf @ wq + bq ; k = kf @ wk + bk ; v = kf @ wv + bv      (wx weight-normed)
  per head h (8 heads, dh=64):
    scores = q_h @ k_h.T / 8 ; strict-causal mask ; softmax ; out_h = attn @ v_h
  out[b] = concat_h(out_h).reshape(32, 32, 512)

Schedule: qf/kf arrive pre-transposed from the host ([C, S] layout), so the
PE runs no input transposes. Inputs stream in as one DMA per 128-row
contraction chunk, split across the two HWDGE queues in consumption order,
so the projection chains accumulate chunk-by-chunk inside the ~13us input
window (the DMA engines run at ~230 GB/s/core aggregate with all 8 cores
loading). A memset-fed PE warm-up covers the pre-data window to hold the
HAM clock at 2.4 GHz. Projections for later head-pairs and each block's
epilogue are emitted as single-instruction filler units drained into the
attention j-loop (with tagged deadline drains for correctness), so the PE
queue never idles while the serial exp (ACT) chain runs. The epilogue
evacuates numerators+denominator to bf16, transposes [d, q] -> [q, d] on
the PE (bf16: 1 cycle/row, into a PSUM ring shared with the projections),
and normalizes on DVE with a [128, 4]-shaped reciprocal.

Numerics: matmul operands are bf16; score accumulation and exp stay fp32
(PSUM accumulates fp32). Softmax runs without max-subtraction (scores are
~N(0,1)). The causal mask is applied multiplicatively after exp (0/1 mask),
matching the reference's -10000 additive mask (exp(-1e4) underflows to 0).
Attention numerators/denominators pass through bf16 during the transpose
(~0.4% rel); row q=0 has an all-zero mask so numerator and denominator are
exactly 0 and the 1e-30 epsilon makes 0/eps = 0, matching the reference's
post-softmax start-mask zeroing. Measured end-to-end relative error vs the
fp32 reference: ~5.5e-3 (absmax-relative).
"""

import os
import sys

import numpy as np

for _p in ("/opt/trn_rl_repo", "/root/.axon_site/_ro/trn_rl_repo"):
    if _p not in sys.path and os.path.isdir(_p):
        sys.path.append(_p)

import concourse.bass as bass
import concourse.mybir as mybir
import concourse.tile as tile

FP = mybir.dt.float32
BF = mybir.dt.bfloat16
AF = mybir.ActivationFunctionType


B = 8
S = 1024
QC, KC, CH = 256, 512, 512
NH, DH = 8, 64
P = 128
NS = S // P    # 8 seq chunks of 128
NAQ = QC // P  # 2 contraction chunks for q proj
NAK = KC // P  # 4 contraction chunks for k/v proj
NP = NH // 2   # 4 head pairs == 4 output-channel chunks of 128
DH1 = DH + 1   # v columns + ones column (softmax denominator)
QW = 512       # q-half width (one PSUM bank of fp32)

N_CORES = 8

_cached_nc = None


def _split_multi_waits(nc, engines=("PE",)):
    """Hoist extra sem-waits onto single-wait NoOps.

    Walrus's CoreV3 codegen rejects PE instructions carrying more than one
    sync wait (setupSyncWait<S3_LW_STRUCT>: "Too many sync wait commands").
    Tile's scheduler freely attaches several waits to one instruction, so
    after scheduling we move all but the last wait of each affected
    instruction onto dedicated same-engine NoOps placed directly before it;
    the engine's sequencer blocks on each NoOp in program order, preserving
    semantics exactly.
    """
    ctr = 0
    for fn in nc.m.functions:
        for blk in fn.blocks:
            new_insts = []
            for inst in blk.instructions:
                si = getattr(inst, "sync_info", None)
                waits = list(si.on_wait) if si is not None and si.on_wait else []
                eng = getattr(inst, "engine", None)
                if (
                    len(waits) > 1
                    and eng is not None
                    and any(e in str(eng) for e in engines)
                ):
                    for w in waits[:-1]:
                        nop = mybir.InstNoOp(
                            name=f"I-wsplit-{ctr}",
                            engine=eng,
                            sync_info=mybir.SyncInfo(on_wait=[w], on_update=[]),
                            bass_nofuse=True,
                        )
                        ctr += 1
                        new_insts.append(nop)
                        nc.inst_map[nop.name] = nop
                    inst.sync_info = mybir.SyncInfo(
                        on_wait=[waits[-1]],
                        on_update=list(si.on_update) if si.on_update else [],
                    )
                new_insts.append(inst)
            blk.instructions[:] = new_insts


def build_module() -> "bass.Bass":
    nc = bass.Bass()

    qfT_d = nc.dram_tensor("qfT", [QC, S], BF, kind="ExternalInput")
    kfT_d = nc.dram_tensor("kfT", [KC, S], BF, kind="ExternalInput")
    wq_d = nc.dram_tensor("wq", [QC, CH], BF, kind="ExternalInput")
    wk_d = nc.dram_tensor("wk", [KC, CH], BF, kind="ExternalInput")
    wv_d = nc.dram_tensor("wv", [KC, CH], BF, kind="ExternalInput")
    bvb_d = nc.dram_tensor("bvb", [P, CH], BF, kind="ExternalInput")
    # packed small constants: bq | bk (fp32 as 2x bf16 slots) | maskT |
    # identity | ones-row
    SM_W = 4 * NP + P + P
    sm_d = nc.dram_tensor("smalls", [P, SM_W], BF, kind="ExternalInput")
    out_d = nc.dram_tensor("out", [S, CH], FP, kind="ExternalOutput")

    with tile.TileContext(nc) as tc:
        with (
            tc.tile_pool(name="const", bufs=1) as cpool,
            tc.tile_pool(name="work", bufs=2) as wpool,
            tc.tile_pool(name="psS", bufs=2, space=bass.MemorySpace.PSUM) as psS,
            tc.tile_pool(name="psO", bufs=2, space=bass.MemorySpace.PSUM) as psO,
            tc.tile_pool(name="psP", bufs=2, space=bass.MemorySpace.PSUM) as psP,
        ):
            # ---- packed constants: one cheap DMA, first on the scalar queue
            sm_sb = cpool.tile([P, SM_W], BF, tag="smalls", name="sm_sb")
            nc.scalar.dma_start(sm_sb[:], sm_d[:])
            bq_sb = sm_sb[:, 0:2 * NP].bitcast(FP)
            bk_sb = sm_sb[:, 2 * NP:4 * NP].bitcast(FP)
            mask_sb = sm_sb[:, 4 * NP:4 * NP + P]
            idb_sb = sm_sb[:, 4 * NP + P:4 * NP + 2 * P]

            # preload the ACT exp table set (~2.7us) during the DMA window
            # instead of stalling the first real exp in the attention phase
            warm_ex = cpool.tile([1, 2], FP, tag="warmex", name="warm_ex")
            nc.scalar.activation(
                warm_ex[:], sm_sb[0:1, 0:2], AF.Exp, scale=0.125
            )
            # PE warm-up: dense dummy matmuls during the input-DMA window keep
            # the HAM activity monitor busy so projections start at 2.4 GHz
            # instead of the cold 1.2 GHz half-clock. The operand tile is
            # memset on GPSIMD so the warm-up does not wait for any DMA.
            wtile = cpool.tile([P, 2 * P], BF, tag="wtile", name="wtile")
            nc.gpsimd.memset(wtile[:], 0.125)
            warm_ps = psP.tile([P, QW], FP, tag="pp", name="warm_ps")
            for _w in range(16):
                nc.tensor.matmul(
                    warm_ps[:, 0:2 * P], wtile[:, 0:P], wtile[:],
                    start=True, stop=True,
                )

            # ---- bulk inputs: one big DMA per tensor, split across the two
            # HWDGE queues in consumption order (kT0 first, then qT0, v).
            kfT_all = cpool.tile([P, NAK * S], BF, tag="kfT", name="kfT_all")
            wv_all = cpool.tile([P, NAK * CH], BF, tag="wv", name="wv_all")
            wk_all = cpool.tile([P, NAK * CH], BF, tag="wk", name="wk_all")
            wq_all = cpool.tile([P, NAQ * CH], BF, tag="wq", name="wq_all")
            qfT_all = cpool.tile([P, NAQ * S], BF, tag="qfT", name="qfT_all")
            bvb_sb = cpool.tile([P, CH], BF, tag="bvb", name="bvb_sb")
            kfT_v = kfT_all[:].rearrange("p (a s) -> p a s", a=NAK)
            wk_v = wk_all[:].rearrange("p (a c) -> p a c", a=NAK)
            wv_v = wv_all[:].rearrange("p (a c) -> p a c", a=NAK)
            wq_v = wq_all[:].rearrange("p (a c) -> p a c", a=NAQ)
            qfT_v = qfT_all[:].rearrange("p (a s) -> p a s", a=NAQ)
            # one DMA per 128-row chunk so the projection chains can start
            # accumulating as each contraction chunk lands, overlapping the
            # k/q/v projections into the input-DMA window
            for a in range(NAK):
                nc.sync.dma_start(kfT_v[:, a], kfT_d[a * P:(a + 1) * P, :])
                nc.scalar.dma_start(wk_v[:, a], wk_d[a * P:(a + 1) * P, :])
            for a in range(NAQ):
                nc.scalar.dma_start(wq_v[:, a], wq_d[a * P:(a + 1) * P, :])
                nc.scalar.dma_start(qfT_v[:, a], qfT_d[a * P:(a + 1) * P, :])
            for a in range(NAK):
                nc.sync.dma_start(wv_v[:, a], wv_d[a * P:(a + 1) * P, :])
            nc.sync.dma_start(bvb_sb[:], bvb_d[:])
            kfT = [kfT_all[:].rearrange("p (a s) -> p a s", a=NAK)[:, a] for a in range(NAK)]
            wv_sb = [wv_all[:].rearrange("p (a c) -> p a c", a=NAK)[:, a] for a in range(NAK)]
            wk_sb = [wk_all[:].rearrange("p (a c) -> p a c", a=NAK)[:, a] for a in range(NAK)]
            wq_sb = [wq_all[:].rearrange("p (a c) -> p a c", a=NAQ)[:, a] for a in range(NAQ)]
            qfT = [qfT_all[:].rearrange("p (a s) -> p a s", a=NAQ)[:, a] for a in range(NAQ)]

            # ---------------- projections ----------------
            # qT/kT in [channel, seq] layout (head-dim on partitions)
            qT = [cpool.tile([P, S], BF, tag=f"qT{c}", name=f"qT{c}") for c in range(NP)]
            kT = [cpool.tile([P, S], BF, tag=f"kT{c}", name=f"kT{c}") for c in range(NP)]
            # v[s, c] per-head blocks of 65 cols (64 data + ones col for the
            # softmax denominator); bias added on DVE during evacuation
            v_sb = [cpool.tile([P, NH * DH1], BF, tag=f"v{si}", name=f"v{si}") for si in range(NS)]
            bvb_view = bvb_sb[:].rearrange("p (h d) -> p h d", h=NH)
            for si in range(NS):
                nc.gpsimd.memset(
                    v_sb[si][:].rearrange("p (h d) -> p h d", h=NH)[:, :, DH:DH1],
                    1.0,
                )

            # Projection emitters, decomposable into single-instruction units
            # so they can drain into PE stalls of the attention loop without
            # delaying the QK->exp chain by more than one matmul.
            def units_qT_half(c, g):
                st = {}

                def mk(a):
                    def u():
                        if a == 0:
                            st["ps"] = psP.tile([P, QW], FP, tag="pp", name="pp_ps")
                        nc.tensor.matmul(
                            st["ps"][:],
                            wq_sb[a][:, c * P:(c + 1) * P],
                            qfT[a][:, g * QW:(g + 1) * QW],
                            start=(a == 0),
                            stop=(a == NAQ - 1),
                            skip_group_check=True,
                        )
                    return u

                def ev():
                    nc.vector.tensor_scalar_add(
                        qT[c][:, g * QW:(g + 1) * QW], st["ps"][:], bq_sb[:, c:c + 1]
                    )

                return [mk(a) for a in range(NAQ)] + [ev]

            def units_kT_half(c, g):
                st = {}

                def mk(a):
                    def u():
                        if a == 0:
                            st["ps"] = psP.tile([P, QW], FP, tag="pp", name="pp_ps")
                        nc.tensor.matmul(
                            st["ps"][:],
                            wk_sb[a][:, c * P:(c + 1) * P],
                            kfT[a][:, g * QW:(g + 1) * QW],
                            start=(a == 0),
                            stop=(a == NAK - 1),
                            skip_group_check=True,
                        )
                    return u

                def ev():
                    nc.vector.tensor_scalar_add(
                        kT[c][:, g * QW:(g + 1) * QW], st["ps"][:], bk_sb[:, c:c + 1]
                    )

                return [mk(a) for a in range(NAK)] + [ev]

            def units_v(si):
                st = {}

                def mk(a):
                    def u():
                        if a == 0:
                            st["ps"] = psP.tile([P, QW], FP, tag="pp", name="pp_ps")
                        nc.tensor.matmul(
                            st["ps"][:, 0:CH],
                            kfT[a][:, si * P:(si + 1) * P],
                            wv_sb[a],
                            start=(a == 0),
                            stop=(a == NAK - 1),
                            skip_group_check=True,
                        )
                    return u

                def ev():
                    v_view = v_sb[si][:].rearrange("p (h d) -> p h d", h=NH)
                    nc.vector.tensor_add(
                        v_view[:, :, 0:DH],
                        st["ps"][:, 0:CH].rearrange("p (h d) -> p h d", h=NH),
                        bvb_view,
                    )

                return [mk(a) for a in range(NAK)] + [ev]

            def emit_now(units):
                for u in units:
                    u()

            # Filler queues: projection + epilogue work for later pairs
            # drains into the attention j-loop of the current pair, so the PE
            # never idles while the ACT exp chain runs. Epilogue units drain
            # with priority (their PSUM accumulators are recycled by the next
            # block); projection units carry tags with drain_until() as the
            # correctness deadline for when a consumer is about to be emitted.
            import collections

            epi_q = collections.deque()
            proj_q = collections.deque()  # (tag, closure)

            def pump(n=1):
                for _ in range(n):
                    if epi_q:
                        epi_q.popleft()()
                    elif proj_q:
                        proj_q.popleft()[1]()

            def drain_until(tag):
                while any(t == tag for t, _ in proj_q):
                    proj_q.popleft()[1]()

            # ---------------- attention: head pairs x q-halves ----------------
            # Heads 2p/2p+1 share qT[p]/kT[p] (rows 0:64 / 64:128). QK for the
            # two heads is row-packed onto the PE array (tile_position), the
            # exp over both heads' scores is one ACT instruction, and the two
            # AV chains interleave to keep PE fed while ACT runs.
            mask_b2 = mask_sb.rearrange("p (o w) -> p o w", o=1).broadcast_to((P, 2, P))

            def attn_pair(p):
                drain_until(f"qh{p}0")
                drain_until(f"kh{p}0")
                tq = qT[p]
                tk = kT[p]
                v_hp = [
                    [v_sb[j][:].rearrange("p (h d) -> p h d", h=NH)[:, 2 * p + idx, :]
                     for idx in range(2)]
                    for j in range(NS)
                ]
                for g in range(2):
                    if g == 1:
                        drain_until(f"qh{p}1")
                    jmax = 4 * (g + 1)
                    outp = [
                        psO.tile([P, QW], FP, tag="outp", name="outp_ps")
                        for _ in range(2)
                    ]

                    def emit_qk(j):
                        off = max(0, j * P - g * QW)
                        sc = psS.tile([P, 2 * QW], FP, tag="sc", name="sc_ps")
                        for idx in range(2):
                            nc.tensor.matmul(
                                sc[:, idx * QW + off:(idx + 1) * QW],
                                tk[idx * DH:(idx + 1) * DH, j * P:(j + 1) * P],
                                tq[idx * DH:(idx + 1) * DH, g * QW + off:(g + 1) * QW],
                                start=True,
                                stop=True,
                                tile_position=(idx * DH, 0),
                            )
                        ex = wpool.tile([P, 2 * QW], BF, tag="ex", name="ex_t", bufs=3)
                        scv = sc[:].rearrange("p (i w) -> p i w", i=2)[:, :, off:QW]
                        exv = ex[:].rearrange("p (i w) -> p i w", i=2)[:, :, off:QW]
                        nc.scalar.activation(exv, scv, AF.Exp, scale=0.125)
                        if g * 4 <= j < g * 4 + 4:  # diagonal block in this half
                            od = j * P - g * QW
                            exd = ex[:].rearrange("p (i w) -> p i w", i=2)[:, :, od:od + P]
                            nc.vector.tensor_mul(exd, exd, mask_b2)
                        return ex

                    def emit_av(j, ex):
                        drain_until(f"v{j}")
                        off = max(0, j * P - g * QW)
                        for idx in range(2):
                            nc.tensor.matmul(
                                outp[idx][0:DH1, off:QW],
                                v_hp[j][idx],
                                ex[:, idx * QW + off:(idx + 1) * QW],
                                start=(j == 0),
                                stop=(j == jmax - 1),
                                skip_group_check=True,
                            )

                    prev_ex = emit_qk(0)
                    pump(4)
                    for j in range(1, jmax):
                        if g == 1 and j == 4:
                            drain_until(f"kh{p}1")
                        cur_ex = emit_qk(j)
                        pump(2)
                        emit_av(j - 1, prev_ex)
                        prev_ex = cur_ex
                    # fill the last exp's latency: the trailing AV waits on
                    # exp(jmax-1) with nothing else in the PE queue otherwise
                    pump(2)
                    emit_av(jmax - 1, prev_ex)

                    # epilogue (as filler units, drained inside the next
                    # attention block): out^T stays in [d, q] layout — the
                    # host does the final [CH,S] -> [S,CH] transpose. The
                    # softmax denominator (outp row 64) is reciprocal'd on
                    # DVE, rank-1-broadcast by the PE into the unused
                    # partitions 64..127 of the same outp bank, and the
                    # numerators are normalized on the (otherwise idle) Pool
                    # engine straight into the bf16 output staging tile.
                    st = {}

                    def mk_evac(idx, outp=outp):
                        def u():
                            outs = wpool.tile([P, QW], BF, tag="outs", name="outs_t", bufs=3)
                            nc.vector.tensor_copy(outs[0:DH1, :], outp[idx][0:DH1, :])
                            st[("outs", idx)] = outs
                        return u

                    def mk_tp(idx):
                        def u():
                            if idx == 0:
                                # shares the projection PSUM ring (one fp32
                                # bank, bitcast to bf16); per-block stride
                                # padded to 66 bf16 elements (132 B) so every
                                # PSUM write offset stays 4-byte aligned
                                tp2 = psP.tile([P, QW], FP, tag="pp", name="tp_ps")
                                st["tpi"] = tp2[:, 0:2 * 2 * (DH1 + 1)].bitcast(
                                    BF
                                ).rearrange("p (i s c) -> p i s c", i=2, c=DH1 + 1)
                            outs = st[("outs", idx)]
                            tpv = st["tpi"][:, idx]
                            for ls in range(4):
                                nc.tensor.transpose(
                                    tpv[:, ls, 0:DH1],
                                    outs[0:DH1, ls * P:(ls + 1) * P],
                                    idb_sb[0:DH1, 0:DH1],
                                )
                        return u

                    def mk_norm(idx):
                        def u():
                            if idx == 0:
                                stage = wpool.tile(
                                    [P, 4 * 2 * DH], FP, tag="stage", name="stage_t", bufs=3
                                )
                                st["stage_v"] = stage[:].rearrange(
                                    "p (s h d) -> p s h d", s=4, h=2
                                )
                            tpv = st["tpi"][:, idx]
                            rc = wpool.tile([P, 4], FP, tag="rc", name="rc_t")
                            rc2 = wpool.tile([P, 4], FP, tag="rc2", name="rc2_t")
                            nc.vector.tensor_scalar_add(
                                rc[:], tpv[:, :, DH:DH1].rearrange("p s o -> p (s o)"), 1e-30
                            )
                            nc.vector.reciprocal(rc2[:], rc[:])
                            rc_b = rc2[:].rearrange("p (s o) -> p s o", o=1).broadcast_to(
                                (P, 4, DH)
                            )
                            nc.vector.tensor_mul(
                                st["stage_v"][:, :, idx, :], tpv[:, :, 0:DH], rc_b
                            )
                        return u

                    def mk_dma(idx, p=p, g=g):
                        def u():
                            nc.sync.dma_start(
                                out_d.rearrange("(s p) c -> p s c", p=P)[
                                    :, 4 * g:4 * (g + 1),
                                    (2 * p + idx) * DH:(2 * p + idx + 1) * DH
                                ],
                                st["stage_v"][:, :, idx, :],
                            )
                        return u

                    epi_q.extend([
                        mk_evac(0), mk_evac(1),
                        mk_tp(0), mk_tp(1),
                        mk_norm(0), mk_norm(1),
                        mk_dma(0), mk_dma(1),
                    ])
                    pump(2)

            # ---------------- schedule ----------------
            # Lead-in: pair-0 projections + v for the g=0 k-blocks; these
            # accumulate chunk-by-chunk as the input DMAs land.
            emit_now(units_kT_half(0, 0))
            emit_now(units_qT_half(0, 0))
            emit_now(units_v(0))
            emit_now(units_v(1))
            for tag, units in (
                [("v2", units_v(2)), ("v3", units_v(3)),
                 ("qh01", units_qT_half(0, 1)), ("kh01", units_kT_half(0, 1))]
                + [(f"v{si}", units_v(si)) for si in range(4, NS)]
                + [t for c in range(1, NP) for t in (
                    (f"qh{c}0", units_qT_half(c, 0)),
                    (f"kh{c}0", units_kT_half(c, 0)),
                    (f"qh{c}1", units_qT_half(c, 1)),
                    (f"kh{c}1", units_kT_half(c, 1)),
                )]
            ):
                for u in units:
                    proj_q.append((tag, u))

            for p in range(NP):
                attn_pair(p)
            # leftovers: the last pair's epilogue units
            while epi_q or proj_q:
                pump()

    _split_multi_waits(
        nc, engines=("PE", "Activation", "DVE", "Pool", "SP", "GPSIMD")
    )
    nc.finalize()
    return nc


def _host_prep(query, key, vq, gq, bq, vk, gk, bk, vv, gv, bv):
    """Weight-norm folding + per-core input maps."""
    f32 = np.float32

    def wn(v, g):
        v = np.asarray(v, f32)
        g = np.asarray(g, f32)
        nrm = np.sqrt(np.sum(v * v, axis=0, dtype=f32), dtype=f32)
        return (v * (g / nrm)).astype(f32)

    wq = wn(vq, gq)
    wk = wn(vk, gk)
    wv = wn(vv, gv)
    bq_r = np.asarray(bq, f32).reshape(NP, P).T
    bk_r = np.asarray(bk, f32).reshape(NP, P).T
    bv_r = np.asarray(bv, f32).reshape(1, CH)
    maskT = np.triu(np.ones((P, P), f32), k=1)  # maskT[k,q] = 1 iff q > k

    import ml_dtypes

    bf16 = ml_dtypes.bfloat16
    query = np.asarray(query, f32)
    key = np.asarray(key, f32)
    wq_b, wk_b, wv_b = wq.astype(bf16), wk.astype(bf16), wv.astype(bf16)
    # packed small constants: bq | bk (fp32 bytes) | maskT | identity
    sm_b = np.concatenate([
        np.ascontiguousarray(bq_r.astype(f32)).view(bf16),
        np.ascontiguousarray(bk_r.astype(f32)).view(bf16),
        maskT.astype(bf16),
        np.eye(P, dtype=f32).astype(bf16),
    ], axis=1)
    bvb = np.broadcast_to(bv_r.astype(bf16), (P, CH)).copy()
    in_maps = []
    for b in range(N_CORES):
        qfT = np.ascontiguousarray(query[b].reshape(S, QC).T).astype(bf16)
        kfT = np.ascontiguousarray(key[b].reshape(S, KC).T).astype(bf16)
        in_maps.append({
            "qfT": qfT,
            "kfT": kfT,
            "wq": wq_b, "wk": wk_b, "wv": wv_b,
            "bvb": bvb,
            "smalls": sm_b,
        })
    return in_maps


def _ensure_ntff_hook():
    """Register the axon NTFF profiling hook if the image lacks the
    antenv.axon_hooks shim module (profiling-only; no effect on results)."""
    import types

    try:
        import antenv.axon_hooks  # noqa: F401
        return
    except ImportError:
        pass
    mod = types.ModuleType("antenv.axon_hooks")
    holder = {"hook": None}
    mod.set_axon_ntff_profile_hook = lambda h: holder.__setitem__("hook", h)
    mod.get_axon_ntff_profile_hook = lambda: holder["hook"]
    sys.modules["antenv.axon_hooks"] = mod
    try:
        import antenv

        antenv.axon_hooks = mod
    except ImportError:
        pass
    try:
        from trn_agent_boot.trn_boot import _ntff_profile_via_ctypes

        mod.set_axon_ntff_profile_hook(
            _ntff_profile_via_ctypes("/opt/axon/libaxon_pjrt.so")
        )
    except Exception:
        pass


def kernel(query, key, vq, gq, bq, vk, gk, bk, vv, gv, bv):
    from concourse.bass_utils import run_bass_kernel_spmd

    global _cached_nc
    if _cached_nc is None:
        _cached_nc = build_module()
    nc = _cached_nc

    in_maps = _host_prep(query, key, vq, gq, bq, vk, gk, bk, vv, gv, bv)
    trace = os.environ.get("KERNEL_TRACE", "0") == "1"
    if trace:
        _ensure_ntff_hook()
    res = run_bass_kernel_spmd(nc, in_maps, list(range(N_CORES)), trace=trace)
    if trace and res.exec_time_ns is not None:
        print(f"HW exec time: {res.exec_time_ns} ns", flush=True)
        kernel.last_exec_time_ns = res.exec_time_ns
    out = np.stack(
        [res.results[b]["out"].reshape(32, 32, CH) for b in range(N_CORES)]
    )
    return out.astype(np.float32)

